# revision 21
# baseline (speedup 1.0000x reference)
"""Trainium2 Bass kernel for nn_AttentionBlock (GroupNorm + MHA + proj + residual).

Sharding: data-parallel over batch — 8 batch elements, one per NeuronCore.
Each core runs the full block for its batch element; no collectives.

Per-core dataflow (c=512, n=1024, heads=8, d=64, groups=32):
  - GroupNorm: per-channel bn_stats/bn_aggr (DVE), group aggregation via a tiny
    f32 matmul with a 1/16 selector matrix, broadcast back to channels via a
    second tiny matmul (PE), normalize fused into one DVE pass producing bf16 y.
  - qkv as matmuls against host-pre-transposed bf16 weights. q/k produced in
    [row, n] layout; v produced directly transposed ([n, vrow]) by swapping
    matmul operands, with a ones-column appended per head (vplus) so the
    attention*V matmul also produces the softmax denominator Z exactly (fp32
    PSUM accumulation).
  - S^T = k_h^T q_h per head in [m, n] layout (K=64 matmuls), exp on ScalarE
    straight out of PSUM into bf16 SBUF.
  - out_h = [v_h^T | 1]^T @ expS^T accumulated over m-tiles -> rows 0..63 are
    the unnormalized output, row 64 is Z. Normalize with reciprocal +
    partition_broadcast + one DVE multiply.
  - proj matmul, then (P + pb_eff) + x fused in one DVE pass.

Host-side algebraic folds (exact):
  - attention scale folded into q weights/bias
  - k bias dropped (row-constant shift is softmax-invariant)
  - v bias folded into proj bias: pb_eff = proj_b + proj_w @ v_b
"""

import sys

for _p in ("/opt/trn_rl_repo", "/root/.axon_site/_ro/trn_rl_repo"):
    if _p not in sys.path:
        sys.path.insert(0, _p)

from contextlib import ExitStack

import ml_dtypes
import numpy as np

import concourse.bass as bass
import concourse.bacc as bacc
import concourse.tile as tile
from concourse import mybir
from concourse.bass_utils import run_bass_kernel_spmd

F32 = mybir.dt.float32
BF16 = mybir.dt.bfloat16
AF = mybir.ActivationFunctionType
OP = mybir.AluOpType

B = 8
C = 512
N = 1024
HEADS = 8
D = 64
GROUPS = 32
GSIZE = C // GROUPS  # 16 channels per group
CT = C // 128  # 4 channel tiles
NT = N // 128  # 8 spatial tiles
W3 = 3 * C
EPS = 1e-5
NCORES = 8


def _build(nc: bass.Bass):
    x = nc.declare_dram_parameter("x", [C, N], F32, isOutput=False)
    qkvwT = nc.declare_dram_parameter("qkvwT", [C, W3], BF16, isOutput=False)
    projwT = nc.declare_dram_parameter("projwT", [C, C], BF16, isOutput=False)
    qb = nc.declare_dram_parameter("qb", [C], F32, isOutput=False)
    pbeff = nc.declare_dram_parameter("pbeff", [C], F32, isOutput=False)
    nw = nc.declare_dram_parameter("nw", [C], F32, isOutput=False)
    nb = nc.declare_dram_parameter("nb", [C], F32, isOutput=False)
    sel = nc.declare_dram_parameter("sel", [CT, 128, GROUPS], F32, isOutput=False)
    selb = nc.declare_dram_parameter("selb", [CT, GROUPS, 128], F32, isOutput=False)
    out = nc.declare_dram_parameter("out", [C, N], F32, isOutput=True)

    with tile.TileContext(nc) as tc, ExitStack() as ctx:
        singles = ctx.enter_context(tc.tile_pool(name="singles", bufs=1))
        small = ctx.enter_context(tc.tile_pool(name="small", bufs=4))
        work = ctx.enter_context(tc.tile_pool(name="work", bufs=2))
        expp = ctx.enter_context(tc.tile_pool(name="expp", bufs=4))
        gn_ctx = ExitStack()
        gnps = gn_ctx.enter_context(tc.tile_pool(name="gnps", bufs=5, space="PSUM"))

        x_sb = singles.tile([128, CT * N], F32)
        y_sb = singles.tile([128, CT * N], BF16)
        q_sb = singles.tile([128, 4 * N], BF16)
        k_sb = singles.tile([128, 4 * N], BF16)
        vplus = singles.tile([128, NT * HEADS * (D + 1)], BF16)  # [nt][h][65]
        av_sb = singles.tile([128, CT * N], BF16)
        wqkv_sb = singles.tile([128, CT * W3], BF16)
        wproj_sb = singles.tile([128, CT * C], BF16)
        bias_sb = singles.tile([128, 16], F32)  # 0:4 qb | 4:8 pbeff | 8:12 nw | 12:16 nb
        sel_sb = singles.tile([128, CT * GROUPS], F32)
        selb_sb = singles.tile([GROUPS, CT * 128], F32)
        zero_sb = singles.tile([128, 1], F32)
        eps_sb = singles.tile([128, 1], F32)
        ab_sb = singles.tile([128, 2 * CT], F32)  # a cols 0..3, b2 cols 4..7

        nc.vector.memset(zero_sb, 0.0)
        nc.vector.memset(eps_sb, EPS)
        nc.vector.memset(vplus, 1.0)

        # x/sel/bias first on the sync queue (groupnorm needs them immediately);
        # the 3.5MB of weights go on the gpsimd DMA queues, gated behind the
        # last x tile so they don't steal HBM bandwidth from the critical path.
        xdmas = []
        for t in range(CT):
            cs = slice(t * 128, (t + 1) * 128)
            xdmas.append(
                nc.sync.dma_start(out=x_sb[:, t * N:(t + 1) * N], in_=x[cs, :])
            )
        for t in range(CT):
            cs = slice(t * 128, (t + 1) * 128)
            w1 = nc.gpsimd.dma_start(
                out=wqkv_sb[:, t * W3:(t + 1) * W3], in_=qkvwT[cs, :]
            )
            w2 = nc.gpsimd.dma_start(
                out=wproj_sb[:, t * C:(t + 1) * C], in_=projwT[cs, :]
            )
            tile.add_dep_helper(w1.ins, xdmas[-1].ins, reason="x before weights")
            tile.add_dep_helper(w2.ins, xdmas[-1].ins, reason="x before weights")

        nc.sync.dma_start(
            out=sel_sb[:].rearrange("p (t g) -> p t g", g=GROUPS),
            in_=sel[:].rearrange("t p g -> p t g"),
        )
        nc.sync.dma_start(
            out=selb_sb[:].rearrange("g (t p) -> g t p", p=128),
            in_=selb[:].rearrange("t g p -> g t p"),
        )
        nc.sync.dma_start(out=bias_sb[:, 0:4], in_=qb[:].rearrange("(t p) -> p t", p=128))
        nc.sync.dma_start(out=bias_sb[:, 4:8], in_=pbeff[:].rearrange("(t p) -> p t", p=128))
        nc.sync.dma_start(out=bias_sb[:, 8:12], in_=nw[:].rearrange("(t p) -> p t", p=128))
        nc.sync.dma_start(out=bias_sb[:, 12:16], in_=nb[:].rearrange("(t p) -> p t", p=128))
        # Each TPB instruction has a single HW wait slot. Absorb the four bias
        # DMA semaphores onto the DVE clock early with tiny touch copies so
        # downstream DVE ops carry at most one (PSUM) wait.
        for j in range(4):
            bt = small.tile([1, 1], F32, tag="btouch", name=f"btouch{j}")
            nc.vector.tensor_copy(out=bt, in_=bias_sb[0:1, 4 * j:4 * j + 1])

        # bf16 staging copies of the selector matrices (entries are exact in
        # bf16); also collapses downstream matmul waits onto the DVE semaphore.
        selbf = singles.tile([128, CT * GROUPS], BF16)
        selbbf = singles.tile([GROUPS, CT * 128], BF16)
        nc.vector.tensor_copy(out=selbf, in_=sel_sb)
        nc.vector.tensor_copy(out=selbbf, in_=selb_sb)

        # ---------------- GroupNorm ----------------
        # Group aggregation uses hi/lo-split bf16 matmuls (exact selector,
        # f32 PSUM accumulation) to recover ~fp32 precision without the
        # fp32-matmul wait-slot limit.
        gps = gnps.tile([GROUPS, 2], F32, tag="gn")  # [E[x], E[x^2]] per group
        for t in range(CT):
            xt = x_sb[:, t * N:(t + 1) * N]
            st = small.tile([128, 2, 6], F32, tag="bn")
            nc.vector.bn_stats(out=st[:, 0, :], in_=xt[:, 0:512])
            nc.vector.bn_stats(out=st[:, 1, :], in_=xt[:, 512:1024])
            mv = small.tile([128, 2], F32, tag="mv")
            nc.vector.bn_aggr(out=mv, in_=st)
            mv2 = small.tile([128, 2], F32, tag="mv2")  # [mean, mean^2 + var]
            nc.vector.tensor_copy(out=mv2[:, 0:1], in_=mv[:, 0:1])
            nc.vector.tensor_scalar(
                out=mv2[:, 1:2], in0=mv[:, 0:1], scalar1=mv[:, 0:1],
                scalar2=mv[:, 1:2], op0=OP.mult, op1=OP.add,
            )
            mv2hi = small.tile([128, 2], BF16, tag="mv2hi")
            nc.vector.tensor_copy(out=mv2hi, in_=mv2)
            mv2lo = small.tile([128, 2], BF16, tag="mv2lo")
            nc.vector.tensor_tensor(out=mv2lo, in0=mv2, in1=mv2hi, op=OP.subtract)
            nc.tensor.matmul(
                gps, lhsT=selbf[:, t * GROUPS:(t + 1) * GROUPS], rhs=mv2hi,
                start=(t == 0), stop=False,
            )
            nc.tensor.matmul(
                gps, lhsT=selbf[:, t * GROUPS:(t + 1) * GROUPS], rhs=mv2lo,
                start=False, stop=(t == CT - 1),
            )
        m2g = small.tile([GROUPS, 1], F32, tag="m2g")
        nc.vector.tensor_scalar(
            out=m2g, in0=gps[:, 0:1], scalar1=gps[:, 0:1], scalar2=None, op0=OP.mult
        )
        vvar = small.tile([GROUPS, 1], F32, tag="vvar")
        nc.vector.tensor_tensor(out=vvar, in0=gps[:, 1:2], in1=m2g, op=OP.subtract)
        sq = small.tile([GROUPS, 1], F32, tag="sq")
        nc.scalar.activation(out=sq, in_=vvar, func=AF.Sqrt, bias=eps_sb[0:GROUPS], scale=1.0)
        gst = small.tile([GROUPS, 2], F32, tag="gst")  # [M, rstd]
        nc.vector.tensor_copy(out=gst[:, 0:1], in_=gps[:, 0:1])
        nc.vector.reciprocal(out=gst[:, 1:2], in_=sq)
        gsthi = small.tile([GROUPS, 2], BF16, tag="gsthi")
        nc.vector.tensor_copy(out=gsthi, in_=gst)
        gstlo = small.tile([GROUPS, 2], BF16, tag="gstlo")
        nc.vector.tensor_tensor(out=gstlo, in0=gst, in1=gsthi, op=OP.subtract)
        # PE toucher: absorb the DVE tick of gsthi/gstlo onto the PE clock so
        # the gbc matmuls carry a single wait (one HW wait slot per inst).
        nc.tensor.ldweights(weights=gstlo[0:1, 0:1])

        for t in range(CT):
            gbc = gnps.tile([128, 2], F32, tag="gn", name=f"gbc{t}")
            nc.tensor.matmul(
                gbc, lhsT=selbbf[0:GROUPS, t * 128:(t + 1) * 128], rhs=gsthi,
                start=True, stop=False,
            )
            nc.tensor.matmul(
                gbc, lhsT=selbbf[0:GROUPS, t * 128:(t + 1) * 128], rhs=gstlo,
                start=False, stop=True,
            )
            at = ab_sb[:, t:t + 1]
            b2t = ab_sb[:, CT + t:CT + t + 1]
            nc.vector.tensor_scalar(
                out=at, in0=bias_sb[:, 8 + t:9 + t], scalar1=gbc[:, 1:2],
                scalar2=None, op0=OP.mult,
            )
            mtmp = small.tile([128, 1], F32, tag="mtmp")
            nc.vector.tensor_scalar(
                out=mtmp, in0=at, scalar1=gbc[:, 0:1], scalar2=None, op0=OP.mult
            )
            nc.vector.tensor_tensor(
                out=b2t, in0=bias_sb[:, 12 + t:13 + t], in1=mtmp, op=OP.subtract
            )
            nc.vector.tensor_scalar(
                out=y_sb[:, t * N:(t + 1) * N], in0=x_sb[:, t * N:(t + 1) * N],
                scalar1=at, scalar2=b2t, op0=OP.mult, op1=OP.add,
            )

        gn_ctx.close()
        ps = ctx.enter_context(tc.tile_pool(name="ps", bufs=2, space="PSUM"))
        psav = ctx.enter_context(tc.tile_pool(name="psav", bufs=2, space="PSUM"))

        # ---------------- QKV ----------------
        # PE touchers: absorb the weight-DMA semaphores before the matmuls.
        for kt in range(CT):
            nc.tensor.ldweights(weights=wqkv_sb[0:1, kt * W3:kt * W3 + 1])
        # q/k in [row, n] layout: row-tiles 0..3 -> q, 4..7 -> k
        for mt in range(8):
            pp = ps.tile([128, N], F32, tag="ps")
            for nh in range(2):
                for kt in range(CT):
                    nc.tensor.matmul(
                        pp[:, nh * 512:(nh + 1) * 512],
                        lhsT=wqkv_sb[:, kt * W3 + mt * 128:kt * W3 + (mt + 1) * 128],
                        rhs=y_sb[:, kt * N + nh * 512:kt * N + (nh + 1) * 512],
                        start=(kt == 0), stop=(kt == CT - 1),
                    )
            if mt < 4:
                nc.vector.tensor_scalar(
                    out=q_sb[:, mt * N:(mt + 1) * N], in0=pp,
                    scalar1=bias_sb[:, mt:mt + 1], scalar2=None, op0=OP.add,
                )
            else:
                km = mt - 4
                nc.vector.tensor_copy(out=k_sb[:, km * N:(km + 1) * N], in_=pp)
        # v directly transposed: [n, vrow], interleaved with ones column per head
        for nt in range(NT):
            vp = ps.tile([128, 512], F32, tag="ps")
            for kt in range(CT):
                nc.tensor.matmul(
                    vp,
                    lhsT=y_sb[:, kt * N + nt * 128:kt * N + nt * 128 + 128],
                    rhs=wqkv_sb[:, kt * W3 + 2 * C:kt * W3 + 3 * C],
                    start=(kt == 0), stop=(kt == CT - 1),
                )
            dst = vplus[:, nt * HEADS * (D + 1):(nt + 1) * HEADS * (D + 1)]
            dst = dst.rearrange("p (h e) -> p h e", e=D + 1)[:, :, 0:D]
            nc.vector.tensor_copy(out=dst, in_=vp.rearrange("p (h e) -> p h e", e=D))

        # ---------------- Attention (per head pair) ----------------
        for pr in range(4):
            heads = ((2 * pr, 0), (2 * pr + 1, 64))
            etiles = {}
            for h, base in heads:
                etiles[h] = expp.tile([128, NT * N], BF16, tag="exp", name=f"exp{h}")
            for mt in range(NT):
                # Interleave the two heads' matmuls (disjoint PE row groups
                # 0-63 / 64-127) so adjacent MMs run concurrently in the array.
                sps = {}
                for h, base in heads:
                    sps[h] = ps.tile([128, N], F32, tag="ps", name=f"sp{h}_{mt}")
                for nh in range(2):
                    for h, base in heads:
                        nc.tensor.matmul(
                            sps[h][:, nh * 512:(nh + 1) * 512],
                            lhsT=k_sb[base:base + 64, pr * N + mt * 128:pr * N + mt * 128 + 128],
                            rhs=q_sb[base:base + 64, pr * N + nh * 512:pr * N + nh * 512 + 512],
                            start=True, stop=True,
                            tile_position=(base, 0),
                        )
                for h, base in heads:
                    nc.scalar.activation(
                        out=etiles[h][:, mt * N:(mt + 1) * N], in_=sps[h],
                        func=AF.Exp, bias=zero_sb, scale=1.0,
                    )
            if pr > 0:
                # Absorb the DVE tick of the previous pair's normalize ops so
                # the av matmuls (which reuse their PSUM slots) wait only on
                # the exp (ACT) semaphore.
                nc.tensor.ldweights(weights=av_sb[0:1, (pr - 1) * N:(pr - 1) * N + 1])
            # Z for both heads collected into a [128, 16] layout (Z[n] at
            # row n//8) so ONE wide reciprocal covers the pair at ~16
            # elems/lane instead of two [1, 1024] single-lane reciprocals.
            zp = small.tile([128, 16], F32, tag="zp", name=f"zp{pr}")
            apns = {}
            for h, base in heads:
                apn = psav.tile([D + 1, N], F32, tag="av", name=f"apn{h}")
                for mt in range(NT):
                    for nh in range(2):
                        nc.tensor.matmul(
                            apn[:, nh * 512:(nh + 1) * 512],
                            lhsT=vplus[:, mt * HEADS * (D + 1) + h * (D + 1):
                                       mt * HEADS * (D + 1) + (h + 1) * (D + 1)],
                            rhs=etiles[h][:, mt * N + nh * 512:mt * N + nh * 512 + 512],
                            start=(mt == 0), stop=(mt == NT - 1),
                        )
                apns[h] = apn
                zrow = small.tile([1, N], F32, tag="zrow", name=f"zrow{h}")
                nc.vector.tensor_copy(out=zrow, in_=apn[D:D + 1, :])
                nc.sync.dma_start(
                    out=zp[:, (h % 2) * 8:(h % 2) * 8 + 8],
                    in_=zrow.rearrange("o (p j) -> o p j", j=8),
                )
            rzp = small.tile([128, 16], F32, tag="rzp", name=f"rzp{pr}")
            nc.vector.reciprocal(out=rzp, in_=zp)
            for h, base in heads:
                rzrow = small.tile([1, N], F32, tag="rzrow", name=f"rzrow{h}")
                nc.sync.dma_start(
                    out=rzrow, in_=rzp[:, (h % 2) * 8:(h % 2) * 8 + 8]
                )
                rzb = work.tile([D, N], F32, tag="rzb")
                nc.gpsimd.partition_broadcast(out_ap=rzb, in_ap=rzrow)
                nc.vector.tensor_tensor(
                    out=av_sb[base:base + 64, pr * N:(pr + 1) * N],
                    in0=apns[h][0:D, :], in1=rzb, op=OP.mult,
                )

        # ---------------- Proj + residual ----------------
        for kt in range(CT):
            nc.tensor.ldweights(weights=wproj_sb[0:1, kt * C:kt * C + 1])
        for ct in range(CT):
            pp = ps.tile([128, N], F32, tag="ps")
            for nh in range(2):
                for kt in range(CT):
                    nc.tensor.matmul(
                        pp[:, nh * 512:(nh + 1) * 512],
                        lhsT=wproj_sb[:, kt * C + ct * 128:kt * C + (ct + 1) * 128],
                        rhs=av_sb[:, kt * N + nh * 512:kt * N + nh * 512 + 512],
                        start=(kt == 0), stop=(kt == CT - 1),
                    )
            ob = work.tile([128, N], F32, tag="osb")
            nc.vector.scalar_tensor_tensor(
                out=ob, in0=pp, scalar=bias_sb[:, 4 + ct:5 + ct],
                in1=x_sb[:, ct * N:(ct + 1) * N], op0=OP.add, op1=OP.add,
            )
            nc.sync.dma_start(out=out[ct * 128:(ct + 1) * 128, :], in_=ob)

    return nc


_CACHE = {}


def _get_nc():
    if "nc" not in _CACHE:
        nc = bacc.Bacc()
        _build(nc)
        nc.finalize()
        _CACHE["nc"] = nc
    return _CACHE["nc"]


def prepare_in_maps(x, norm_w, norm_b, qkv_w, qkv_b, proj_w, proj_b):
    x = np.asarray(x, np.float32)
    norm_w = np.asarray(norm_w, np.float32)
    norm_b = np.asarray(norm_b, np.float32)
    qkv_w = np.asarray(qkv_w, np.float32).copy()
    qkv_b = np.asarray(qkv_b, np.float32).copy()
    proj_w = np.asarray(proj_w, np.float32)
    proj_b = np.asarray(proj_b, np.float32)

    scale = D ** -0.5
    qkv_w[:C] *= scale
    qbias = (qkv_b[:C] * scale).astype(np.float32)
    vbias = qkv_b[2 * C:3 * C]
    qkvwT = np.ascontiguousarray(qkv_w.T).astype(ml_dtypes.bfloat16)
    projwT = np.ascontiguousarray(proj_w.T).astype(ml_dtypes.bfloat16)
    pb_eff = (proj_b + proj_w @ vbias).astype(np.float32)

    sel = np.zeros([CT, 128, GROUPS], np.float32)
    selb = np.zeros([CT, GROUPS, 128], np.float32)
    for t in range(CT):
        for p in range(128):
            g = (t * 128 + p) // GSIZE
            sel[t, p, g] = 1.0 / GSIZE
            selb[t, g, p] = 1.0
    shared = dict(
        qkvwT=qkvwT, projwT=projwT, qb=qbias, pbeff=pb_eff,
        nw=norm_w, nb=norm_b, sel=sel, selb=selb,
    )
    return [
        dict(x=np.ascontiguousarray(x[i].reshape(C, N)), **shared)
        for i in range(x.shape[0])
    ]


def run(in_maps, trace=False, **kwargs):
    return run_bass_kernel_spmd(
        _get_nc(), in_maps, core_ids=list(range(NCORES)), trace=trace, **kwargs
    )


def kernel(x, norm_w, norm_b, qkv_w, qkv_b, proj_w, proj_b):
    in_maps = prepare_in_maps(x, norm_w, norm_b, qkv_w, qkv_b, proj_w, proj_b)
    res = run(in_maps)
    b, c, h, w = np.asarray(x).shape
    return np.stack(
        [res.results[i]["out"].reshape(c, h, w) for i in range(b)]
    ).astype(np.float32)


# revision 22
# speedup vs baseline: 1.0114x; 1.0114x over previous
"""Trainium2 Bass kernel for nn_AttentionBlock (GroupNorm + MHA + proj + residual).

Sharding: data-parallel over batch — 8 batch elements, one per NeuronCore.
Each core runs the full block for its batch element; no collectives.

Per-core dataflow (c=512, n=1024, heads=8, d=64, groups=32):
  - GroupNorm: per-channel bn_stats/bn_aggr (DVE), group aggregation via a tiny
    f32 matmul with a 1/16 selector matrix, broadcast back to channels via a
    second tiny matmul (PE), normalize fused into one DVE pass producing bf16 y.
  - qkv as matmuls against host-pre-transposed bf16 weights. q/k produced in
    [row, n] layout; v produced directly transposed ([n, vrow]) by swapping
    matmul operands, with a ones-column appended per head (vplus) so the
    attention*V matmul also produces the softmax denominator Z exactly (fp32
    PSUM accumulation).
  - S^T = k_h^T q_h per head in [m, n] layout (K=64 matmuls), exp on ScalarE
    straight out of PSUM into bf16 SBUF.
  - out_h = [v_h^T | 1]^T @ expS^T accumulated over m-tiles -> rows 0..63 are
    the unnormalized output, row 64 is Z. Normalize with reciprocal +
    partition_broadcast + one DVE multiply.
  - proj matmul, then (P + pb_eff) + x fused in one DVE pass.

Host-side algebraic folds (exact):
  - attention scale folded into q weights/bias
  - k bias dropped (row-constant shift is softmax-invariant)
  - v bias folded into proj bias: pb_eff = proj_b + proj_w @ v_b
"""

import sys

for _p in ("/opt/trn_rl_repo", "/root/.axon_site/_ro/trn_rl_repo"):
    if _p not in sys.path:
        sys.path.insert(0, _p)

from contextlib import ExitStack

import ml_dtypes
import numpy as np

import concourse.bass as bass
import concourse.bacc as bacc
import concourse.tile as tile
from concourse import mybir
from concourse.bass_utils import run_bass_kernel_spmd

F32 = mybir.dt.float32
BF16 = mybir.dt.bfloat16
AF = mybir.ActivationFunctionType
OP = mybir.AluOpType

B = 8
C = 512
N = 1024
HEADS = 8
D = 64
GROUPS = 32
GSIZE = C // GROUPS  # 16 channels per group
CT = C // 128  # 4 channel tiles
NT = N // 128  # 8 spatial tiles
W3 = 3 * C
EPS = 1e-5
NCORES = 8


def _build(nc: bass.Bass):
    x = nc.declare_dram_parameter("x", [C, N], F32, isOutput=False)
    qkvwT = nc.declare_dram_parameter("qkvwT", [C, W3], BF16, isOutput=False)
    projwT = nc.declare_dram_parameter("projwT", [C, C], BF16, isOutput=False)
    qb = nc.declare_dram_parameter("qb", [C], F32, isOutput=False)
    pbeff = nc.declare_dram_parameter("pbeff", [C], F32, isOutput=False)
    nw = nc.declare_dram_parameter("nw", [C], F32, isOutput=False)
    nb = nc.declare_dram_parameter("nb", [C], F32, isOutput=False)
    sel = nc.declare_dram_parameter("sel", [CT, 128, GROUPS], F32, isOutput=False)
    selb = nc.declare_dram_parameter("selb", [CT, GROUPS, 128], F32, isOutput=False)
    out = nc.declare_dram_parameter("out", [C, N], F32, isOutput=True)

    with tile.TileContext(nc) as tc, ExitStack() as ctx:
        singles = ctx.enter_context(tc.tile_pool(name="singles", bufs=1))
        small = ctx.enter_context(tc.tile_pool(name="small", bufs=4))
        work = ctx.enter_context(tc.tile_pool(name="work", bufs=2))
        expp = ctx.enter_context(tc.tile_pool(name="expp", bufs=4))
        gn_ctx = ExitStack()
        gnps = gn_ctx.enter_context(tc.tile_pool(name="gnps", bufs=5, space="PSUM"))

        x_sb = singles.tile([128, CT * N], F32)
        y_sb = singles.tile([128, CT * N], BF16)
        q_sb = singles.tile([128, 4 * N], BF16)
        k_sb = singles.tile([128, 4 * N], BF16)
        vplus = singles.tile([128, NT * HEADS * (D + 1)], BF16)  # [nt][h][65]
        av_sb = singles.tile([128, CT * N], BF16)
        wqkv_sb = singles.tile([128, CT * W3], BF16)
        wproj_sb = singles.tile([128, CT * C], BF16)
        bias_sb = singles.tile([128, 16], F32)  # 0:4 qb | 4:8 pbeff | 8:12 nw | 12:16 nb
        sel_sb = singles.tile([128, CT * GROUPS], F32)
        selb_sb = singles.tile([GROUPS, CT * 128], F32)
        zero_sb = singles.tile([128, 1], F32)
        eps_sb = singles.tile([128, 1], F32)
        ab_sb = singles.tile([128, 2 * CT], F32)  # a cols 0..3, b2 cols 4..7

        nc.vector.memset(zero_sb, 0.0)
        nc.vector.memset(eps_sb, EPS)
        nc.vector.memset(vplus, 1.0)

        # x/sel/bias first on the sync queue (groupnorm needs them immediately);
        # the 3.5MB of weights go on the gpsimd DMA queues, gated behind the
        # last x tile so they don't steal HBM bandwidth from the critical path.
        xdmas = []
        for t in range(CT):
            cs = slice(t * 128, (t + 1) * 128)
            xdmas.append(
                nc.sync.dma_start(out=x_sb[:, t * N:(t + 1) * N], in_=x[cs, :])
            )
        for t in range(CT):
            cs = slice(t * 128, (t + 1) * 128)
            w1 = nc.gpsimd.dma_start(
                out=wqkv_sb[:, t * W3:(t + 1) * W3], in_=qkvwT[cs, :]
            )
            w2 = nc.gpsimd.dma_start(
                out=wproj_sb[:, t * C:(t + 1) * C], in_=projwT[cs, :]
            )
            tile.add_dep_helper(w1.ins, xdmas[-1].ins, reason="x before weights")
            tile.add_dep_helper(w2.ins, xdmas[-1].ins, reason="x before weights")

        nc.sync.dma_start(
            out=sel_sb[:].rearrange("p (t g) -> p t g", g=GROUPS),
            in_=sel[:].rearrange("t p g -> p t g"),
        )
        nc.sync.dma_start(
            out=selb_sb[:].rearrange("g (t p) -> g t p", p=128),
            in_=selb[:].rearrange("t g p -> g t p"),
        )
        nc.sync.dma_start(out=bias_sb[:, 0:4], in_=qb[:].rearrange("(t p) -> p t", p=128))
        nc.sync.dma_start(out=bias_sb[:, 4:8], in_=pbeff[:].rearrange("(t p) -> p t", p=128))
        nc.sync.dma_start(out=bias_sb[:, 8:12], in_=nw[:].rearrange("(t p) -> p t", p=128))
        nc.sync.dma_start(out=bias_sb[:, 12:16], in_=nb[:].rearrange("(t p) -> p t", p=128))
        # Each TPB instruction has a single HW wait slot. Absorb the four bias
        # DMA semaphores onto the DVE clock early with tiny touch copies so
        # downstream DVE ops carry at most one (PSUM) wait.
        for j in range(4):
            bt = small.tile([1, 1], F32, tag="btouch", name=f"btouch{j}")
            nc.vector.tensor_copy(out=bt, in_=bias_sb[0:1, 4 * j:4 * j + 1])

        # bf16 staging copies of the selector matrices (entries are exact in
        # bf16); also collapses downstream matmul waits onto the DVE semaphore.
        selbf = singles.tile([128, CT * GROUPS], BF16)
        selbbf = singles.tile([GROUPS, CT * 128], BF16)
        nc.vector.tensor_copy(out=selbf, in_=sel_sb)
        nc.vector.tensor_copy(out=selbbf, in_=selb_sb)

        # ---------------- GroupNorm ----------------
        # Group aggregation uses hi/lo-split bf16 matmuls (exact selector,
        # f32 PSUM accumulation) to recover ~fp32 precision without the
        # fp32-matmul wait-slot limit.
        gps = gnps.tile([GROUPS, 2], F32, tag="gn")  # [E[x], E[x^2]] per group
        for t in range(CT):
            xt = x_sb[:, t * N:(t + 1) * N]
            st = small.tile([128, 2, 6], F32, tag="bn")
            nc.vector.bn_stats(out=st[:, 0, :], in_=xt[:, 0:512])
            nc.vector.bn_stats(out=st[:, 1, :], in_=xt[:, 512:1024])
            mv = small.tile([128, 2], F32, tag="mv")
            nc.vector.bn_aggr(out=mv, in_=st)
            mv2 = small.tile([128, 2], F32, tag="mv2")  # [mean, mean^2 + var]
            nc.vector.tensor_copy(out=mv2[:, 0:1], in_=mv[:, 0:1])
            nc.vector.tensor_scalar(
                out=mv2[:, 1:2], in0=mv[:, 0:1], scalar1=mv[:, 0:1],
                scalar2=mv[:, 1:2], op0=OP.mult, op1=OP.add,
            )
            mv2hi = small.tile([128, 2], BF16, tag="mv2hi")
            nc.vector.tensor_copy(out=mv2hi, in_=mv2)
            mv2lo = small.tile([128, 2], BF16, tag="mv2lo")
            nc.vector.tensor_tensor(out=mv2lo, in0=mv2, in1=mv2hi, op=OP.subtract)
            nc.tensor.matmul(
                gps, lhsT=selbf[:, t * GROUPS:(t + 1) * GROUPS], rhs=mv2hi,
                start=(t == 0), stop=False,
            )
            nc.tensor.matmul(
                gps, lhsT=selbf[:, t * GROUPS:(t + 1) * GROUPS], rhs=mv2lo,
                start=False, stop=(t == CT - 1),
            )
        m2g = small.tile([GROUPS, 1], F32, tag="m2g")
        nc.vector.tensor_scalar(
            out=m2g, in0=gps[:, 0:1], scalar1=gps[:, 0:1], scalar2=None, op0=OP.mult
        )
        vvar = small.tile([GROUPS, 1], F32, tag="vvar")
        nc.vector.tensor_tensor(out=vvar, in0=gps[:, 1:2], in1=m2g, op=OP.subtract)
        sq = small.tile([GROUPS, 1], F32, tag="sq")
        nc.scalar.activation(out=sq, in_=vvar, func=AF.Sqrt, bias=eps_sb[0:GROUPS], scale=1.0)
        gst = small.tile([GROUPS, 2], F32, tag="gst")  # [M, rstd]
        nc.vector.tensor_copy(out=gst[:, 0:1], in_=gps[:, 0:1])
        nc.vector.reciprocal(out=gst[:, 1:2], in_=sq)
        gsthi = small.tile([GROUPS, 2], BF16, tag="gsthi")
        nc.vector.tensor_copy(out=gsthi, in_=gst)
        gstlo = small.tile([GROUPS, 2], BF16, tag="gstlo")
        nc.vector.tensor_tensor(out=gstlo, in0=gst, in1=gsthi, op=OP.subtract)
        # PE toucher: absorb the DVE tick of gsthi/gstlo onto the PE clock so
        # the gbc matmuls carry a single wait (one HW wait slot per inst).
        nc.tensor.ldweights(weights=gstlo[0:1, 0:1])

        for t in range(CT):
            gbc = gnps.tile([128, 2], F32, tag="gn", name=f"gbc{t}")
            nc.tensor.matmul(
                gbc, lhsT=selbbf[0:GROUPS, t * 128:(t + 1) * 128], rhs=gsthi,
                start=True, stop=False,
            )
            nc.tensor.matmul(
                gbc, lhsT=selbbf[0:GROUPS, t * 128:(t + 1) * 128], rhs=gstlo,
                start=False, stop=True,
            )
            at = ab_sb[:, t:t + 1]
            b2t = ab_sb[:, CT + t:CT + t + 1]
            nc.vector.tensor_scalar(
                out=at, in0=bias_sb[:, 8 + t:9 + t], scalar1=gbc[:, 1:2],
                scalar2=None, op0=OP.mult,
            )
            mtmp = small.tile([128, 1], F32, tag="mtmp")
            nc.vector.tensor_scalar(
                out=mtmp, in0=at, scalar1=gbc[:, 0:1], scalar2=None, op0=OP.mult
            )
            nc.vector.tensor_tensor(
                out=b2t, in0=bias_sb[:, 12 + t:13 + t], in1=mtmp, op=OP.subtract
            )
            nc.vector.tensor_scalar(
                out=y_sb[:, t * N:(t + 1) * N], in0=x_sb[:, t * N:(t + 1) * N],
                scalar1=at, scalar2=b2t, op0=OP.mult, op1=OP.add,
            )

        gn_ctx.close()
        ps = ctx.enter_context(tc.tile_pool(name="ps", bufs=2, space="PSUM"))
        psav = ctx.enter_context(tc.tile_pool(name="psav", bufs=2, space="PSUM"))

        # ---------------- QKV ----------------
        # PE touchers: absorb the weight-DMA semaphores before the matmuls.
        for kt in range(CT):
            nc.tensor.ldweights(weights=wqkv_sb[0:1, kt * W3:kt * W3 + 1])
        # q/k in [row, n] layout: row-tiles 0..3 -> q, 4..7 -> k
        for mt in range(8):
            pp = ps.tile([128, N], F32, tag="ps")
            for nh in range(2):
                for kt in range(CT):
                    nc.tensor.matmul(
                        pp[:, nh * 512:(nh + 1) * 512],
                        lhsT=wqkv_sb[:, kt * W3 + mt * 128:kt * W3 + (mt + 1) * 128],
                        rhs=y_sb[:, kt * N + nh * 512:kt * N + (nh + 1) * 512],
                        start=(kt == 0), stop=(kt == CT - 1),
                    )
            if mt < 4:
                nc.vector.tensor_scalar(
                    out=q_sb[:, mt * N:(mt + 1) * N], in0=pp,
                    scalar1=bias_sb[:, mt:mt + 1], scalar2=None, op0=OP.add,
                )
            else:
                km = mt - 4
                nc.vector.tensor_copy(out=k_sb[:, km * N:(km + 1) * N], in_=pp)
        # v directly transposed: [n, vrow], interleaved with ones column per head
        for nt in range(NT):
            vp = ps.tile([128, 512], F32, tag="ps")
            for kt in range(CT):
                nc.tensor.matmul(
                    vp,
                    lhsT=y_sb[:, kt * N + nt * 128:kt * N + nt * 128 + 128],
                    rhs=wqkv_sb[:, kt * W3 + 2 * C:kt * W3 + 3 * C],
                    start=(kt == 0), stop=(kt == CT - 1),
                )
            dst = vplus[:, nt * HEADS * (D + 1):(nt + 1) * HEADS * (D + 1)]
            dst = dst.rearrange("p (h e) -> p h e", e=D + 1)[:, :, 0:D]
            nc.vector.tensor_copy(out=dst, in_=vp.rearrange("p (h e) -> p h e", e=D))

        # ---------------- Attention (software-pipelined over head pairs) ----
        # Iteration pr emits pair pr's qk+exp chunks interleaved per m-tile
        # with pair pr-1's attention*V matmuls, so the PE keeps feeding ACT
        # new S chunks while it drains the previous pair — both engines stay
        # busy and the PE avoids long HAM-rethrottling gaps.
        prev = None  # (pr, heads, etiles, apns)
        for pr in range(5):
            if pr < 4:
                heads = ((2 * pr, 0), (2 * pr + 1, 64))
                etiles = {}
                for h, base in heads:
                    etiles[h] = expp.tile(
                        [128, NT * N], BF16, tag="exp", name=f"exp{h}"
                    )
                apns = {}
                for h, base in heads:
                    apns[h] = psav.tile([D + 1, N], F32, tag="av", name=f"apn{h}")
            for mt in range(NT):
                if pr < 4:
                    # Two heads' qk interleaved: disjoint PE row groups
                    # (0-63 / 64-127) -> adjacent MMs run concurrently.
                    sps = {}
                    for h, base in heads:
                        sps[h] = ps.tile([128, N], F32, tag="ps", name=f"sp{h}_{mt}")
                    for nh in range(2):
                        for h, base in heads:
                            nc.tensor.matmul(
                                sps[h][:, nh * 512:(nh + 1) * 512],
                                lhsT=k_sb[base:base + 64, pr * N + mt * 128:pr * N + mt * 128 + 128],
                                rhs=q_sb[base:base + 64, pr * N + nh * 512:pr * N + nh * 512 + 512],
                                start=True, stop=True,
                                tile_position=(base, 0),
                            )
                    for h, base in heads:
                        nc.scalar.activation(
                            out=etiles[h][:, mt * N:(mt + 1) * N], in_=sps[h],
                            func=AF.Exp, bias=zero_sb, scale=1.0,
                        )
                if prev is not None:
                    p_pr, p_heads, p_etiles, p_apns = prev
                    for h, base in p_heads:
                        for nh in range(2):
                            nc.tensor.matmul(
                                p_apns[h][:, nh * 512:(nh + 1) * 512],
                                lhsT=vplus[:, mt * HEADS * (D + 1) + h * (D + 1):
                                           mt * HEADS * (D + 1) + (h + 1) * (D + 1)],
                                rhs=p_etiles[h][:, mt * N + nh * 512:mt * N + nh * 512 + 512],
                                start=(mt == 0), stop=(mt == NT - 1),
                            )
            if prev is not None:
                # Softmax denominators for the drained pair: collect both Z
                # rows into a [128, 16] layout so ONE wide reciprocal covers
                # the pair (~16 elems/lane), then broadcast and normalize.
                p_pr, p_heads, p_etiles, p_apns = prev
                zp = small.tile([128, 16], F32, tag="zp", name=f"zp{p_pr}")
                for h, base in p_heads:
                    zrow = small.tile([1, N], F32, tag="zrow", name=f"zrow{h}")
                    nc.vector.tensor_copy(out=zrow, in_=p_apns[h][D:D + 1, :])
                    nc.sync.dma_start(
                        out=zp[:, (h % 2) * 8:(h % 2) * 8 + 8],
                        in_=zrow.rearrange("o (p j) -> o p j", j=8),
                    )
                rzp = small.tile([128, 16], F32, tag="rzp", name=f"rzp{p_pr}")
                nc.vector.reciprocal(out=rzp, in_=zp)
                for h, base in p_heads:
                    rzrow = small.tile([1, N], F32, tag="rzrow", name=f"rzrow{h}")
                    nc.sync.dma_start(
                        out=rzrow, in_=rzp[:, (h % 2) * 8:(h % 2) * 8 + 8]
                    )
                    rzb = work.tile([D, N], F32, tag="rzb")
                    nc.gpsimd.partition_broadcast(out_ap=rzb, in_ap=rzrow)
                    nc.vector.tensor_tensor(
                        out=av_sb[base:base + 64, p_pr * N:(p_pr + 1) * N],
                        in0=p_apns[h][0:D, :], in1=rzb, op=OP.mult,
                    )
            prev = (pr, heads, etiles, apns) if pr < 4 else None

        # ---------------- Proj + residual ----------------
        for kt in range(CT):
            nc.tensor.ldweights(weights=wproj_sb[0:1, kt * C:kt * C + 1])
        for ct in range(CT):
            pp = ps.tile([128, N], F32, tag="ps")
            for nh in range(2):
                for kt in range(CT):
                    nc.tensor.matmul(
                        pp[:, nh * 512:(nh + 1) * 512],
                        lhsT=wproj_sb[:, kt * C + ct * 128:kt * C + (ct + 1) * 128],
                        rhs=av_sb[:, kt * N + nh * 512:kt * N + nh * 512 + 512],
                        start=(kt == 0), stop=(kt == CT - 1),
                    )
            ob = work.tile([128, N], F32, tag="osb")
            nc.vector.scalar_tensor_tensor(
                out=ob, in0=pp, scalar=bias_sb[:, 4 + ct:5 + ct],
                in1=x_sb[:, ct * N:(ct + 1) * N], op0=OP.add, op1=OP.add,
            )
            nc.sync.dma_start(out=out[ct * 128:(ct + 1) * 128, :], in_=ob)

    return nc


_CACHE = {}


def _get_nc():
    if "nc" not in _CACHE:
        nc = bacc.Bacc()
        _build(nc)
        nc.finalize()
        _CACHE["nc"] = nc
    return _CACHE["nc"]


def prepare_in_maps(x, norm_w, norm_b, qkv_w, qkv_b, proj_w, proj_b):
    x = np.asarray(x, np.float32)
    norm_w = np.asarray(norm_w, np.float32)
    norm_b = np.asarray(norm_b, np.float32)
    qkv_w = np.asarray(qkv_w, np.float32).copy()
    qkv_b = np.asarray(qkv_b, np.float32).copy()
    proj_w = np.asarray(proj_w, np.float32)
    proj_b = np.asarray(proj_b, np.float32)

    scale = D ** -0.5
    qkv_w[:C] *= scale
    qbias = (qkv_b[:C] * scale).astype(np.float32)
    vbias = qkv_b[2 * C:3 * C]
    qkvwT = np.ascontiguousarray(qkv_w.T).astype(ml_dtypes.bfloat16)
    projwT = np.ascontiguousarray(proj_w.T).astype(ml_dtypes.bfloat16)
    pb_eff = (proj_b + proj_w @ vbias).astype(np.float32)

    sel = np.zeros([CT, 128, GROUPS], np.float32)
    selb = np.zeros([CT, GROUPS, 128], np.float32)
    for t in range(CT):
        for p in range(128):
            g = (t * 128 + p) // GSIZE
            sel[t, p, g] = 1.0 / GSIZE
            selb[t, g, p] = 1.0
    shared = dict(
        qkvwT=qkvwT, projwT=projwT, qb=qbias, pbeff=pb_eff,
        nw=norm_w, nb=norm_b, sel=sel, selb=selb,
    )
    return [
        dict(x=np.ascontiguousarray(x[i].reshape(C, N)), **shared)
        for i in range(x.shape[0])
    ]


def run(in_maps, trace=False, **kwargs):
    return run_bass_kernel_spmd(
        _get_nc(), in_maps, core_ids=list(range(NCORES)), trace=trace, **kwargs
    )


def kernel(x, norm_w, norm_b, qkv_w, qkv_b, proj_w, proj_b):
    in_maps = prepare_in_maps(x, norm_w, norm_b, qkv_w, qkv_b, proj_w, proj_b)
    res = run(in_maps)
    b, c, h, w = np.asarray(x).shape
    return np.stack(
        [res.results[i]["out"].reshape(c, h, w) for i in range(b)]
    ).astype(np.float32)


# revision 23
# speedup vs baseline: 1.0188x; 1.0073x over previous
"""Trainium2 Bass kernel for nn_AttentionBlock (GroupNorm + MHA + proj + residual).

Sharding: data-parallel over batch — 8 batch elements, one per NeuronCore.
Each core runs the full block for its batch element; no collectives.

Per-core dataflow (c=512, n=1024, heads=8, d=64, groups=32):
  - GroupNorm per 128-channel tile (groups never cross tiles): bn_stats/
    bn_aggr (DVE), group aggregation + broadcast-back via tiny hi/lo-split
    bf16 matmuls (exact selectors, f32 PSUM), normalize fused into one DVE
    pass producing bf16 y.
  - qkv as matmuls against host-pre-transposed bf16 weights. q/k in [row, n]
    layout; v produced directly transposed ([n, vrow]) by swapping matmul
    operands, with a ones-column per head (vplus) so the attention*V matmul
    also produces the softmax denominator Z exactly in f32 PSUM.
  - S^T = k_h^T q_h per head in [m, n] layout (K=64 matmuls, head pairs on
    disjoint PE row groups run concurrently), exp on ScalarE straight from
    PSUM into bf16 SBUF. ScalarE is the steady-state bottleneck, so the
    emission is software-pipelined: pair p's qk/exp interleaves with pair
    p-1's attention*V matmuls, and most of the qkv/vT matmuls are deferred
    into pair 0's loop so exp starts as early as possible.
  - Z normalization: both heads' Z rows gathered into a [128, 16] layout so
    one wide reciprocal covers a pair (~16 elems/lane), then DRAM-bounce
    DMA broadcast (no gpsimd drains) and one DVE multiply per head.
  - proj matmul, then (P + pb_eff) + x fused in one DVE pass.

Host-side algebraic folds (exact):
  - attention scale folded into q weights/bias
  - k bias dropped (row-constant shift is softmax-invariant)
  - v bias folded into proj bias: pb_eff = proj_b + proj_w @ v_b
"""

import sys

for _p in ("/opt/trn_rl_repo", "/root/.axon_site/_ro/trn_rl_repo"):
    if _p not in sys.path:
        sys.path.insert(0, _p)

from contextlib import ExitStack

import ml_dtypes
import numpy as np

import concourse.bass as bass
import concourse.bacc as bacc
import concourse.tile as tile
from concourse import mybir
from concourse.bass_utils import run_bass_kernel_spmd

F32 = mybir.dt.float32
BF16 = mybir.dt.bfloat16
AF = mybir.ActivationFunctionType
OP = mybir.AluOpType

B = 8
C = 512
N = 1024
HEADS = 8
D = 64
GROUPS = 32
GSIZE = C // GROUPS  # 16 channels per group
CT = C // 128  # 4 channel tiles
GPT = GROUPS // CT  # 8 groups per channel tile
NT = N // 128  # 8 spatial tiles
W3 = 3 * C
EPS = 1e-5
NCORES = 8
VW = D + 1  # v columns per head incl. ones column


def _build(nc: bass.Bass):
    x = nc.declare_dram_parameter("x", [C, N], F32, isOutput=False)
    qkvwT = nc.declare_dram_parameter("qkvwT", [C, W3], BF16, isOutput=False)
    projwT = nc.declare_dram_parameter("projwT", [C, C], BF16, isOutput=False)
    qb = nc.declare_dram_parameter("qb", [C], F32, isOutput=False)
    pbeff = nc.declare_dram_parameter("pbeff", [C], F32, isOutput=False)
    nw = nc.declare_dram_parameter("nw", [C], F32, isOutput=False)
    nb = nc.declare_dram_parameter("nb", [C], F32, isOutput=False)
    sel = nc.declare_dram_parameter("sel", [CT, 128, GPT], F32, isOutput=False)
    selb = nc.declare_dram_parameter("selb", [CT, GPT, 128], F32, isOutput=False)
    out = nc.declare_dram_parameter("out", [C, N], F32, isOutput=True)

    with tile.TileContext(nc) as tc, ExitStack() as ctx:
        singles = ctx.enter_context(tc.tile_pool(name="singles", bufs=1))
        small = ctx.enter_context(tc.tile_pool(name="small", bufs=4))
        work = ctx.enter_context(tc.tile_pool(name="work", bufs=2))
        expp = ctx.enter_context(tc.tile_pool(name="expp", bufs=4))
        drp = ctx.enter_context(tc.tile_pool(name="drp", bufs=4, space="DRAM"))
        gn_ctx = ExitStack()
        gnps = gn_ctx.enter_context(tc.tile_pool(name="gnps", bufs=4, space="PSUM"))

        x_sb = singles.tile([128, CT * N], F32)
        y_sb = singles.tile([128, CT * N], BF16)
        q_sb = singles.tile([128, 4 * N], BF16)
        k_sb = singles.tile([128, 4 * N], BF16)
        vplus = singles.tile([128, NT * HEADS * VW], BF16)  # [nt][h][65]
        av_sb = singles.tile([128, CT * N], BF16)
        wqkv_sb = singles.tile([128, CT * W3], BF16)
        wproj_sb = singles.tile([128, CT * C], BF16)
        bias_sb = singles.tile([128, 16], F32)  # 0:4 qb | 4:8 pbeff | 8:12 nw | 12:16 nb
        sel_sb = singles.tile([128, CT * GPT], F32)
        selb_sb = singles.tile([GPT, CT * 128], F32)
        zero_sb = singles.tile([128, 1], F32)
        eps_sb = singles.tile([128, 1], F32)
        ab_sb = singles.tile([128, 2 * CT], F32)  # a cols 0..3, b2 cols 4..7

        nc.vector.memset(zero_sb, 0.0)
        nc.vector.memset(eps_sb, EPS)
        nc.vector.memset(vplus, 1.0)

        # x first (groupnorm needs it immediately); 3.5MB of weights on the
        # gpsimd DMA queues gated behind the first two x tiles so they don't
        # starve the critical path but still arrive before the qkv matmuls.
        xdmas = []
        for t in range(CT):
            cs = slice(t * 128, (t + 1) * 128)
            xdmas.append(
                nc.sync.dma_start(out=x_sb[:, t * N:(t + 1) * N], in_=x[cs, :])
            )
        for t in range(CT):
            cs = slice(t * 128, (t + 1) * 128)
            w1 = nc.gpsimd.dma_start(
                out=wqkv_sb[:, t * W3:(t + 1) * W3], in_=qkvwT[cs, :]
            )
            w2 = nc.gpsimd.dma_start(
                out=wproj_sb[:, t * C:(t + 1) * C], in_=projwT[cs, :]
            )
            tile.add_dep_helper(w1.ins, xdmas[1].ins, reason="x before weights")
            tile.add_dep_helper(w2.ins, xdmas[1].ins, reason="x before weights")
        nc.sync.dma_start(
            out=sel_sb[:].rearrange("p (t g) -> p t g", g=GPT),
            in_=sel[:].rearrange("t p g -> p t g"),
        )
        nc.sync.dma_start(
            out=selb_sb[:].rearrange("g (t p) -> g t p", p=128),
            in_=selb[:].rearrange("t g p -> g t p"),
        )
        nc.sync.dma_start(out=bias_sb[:, 0:4], in_=qb[:].rearrange("(t p) -> p t", p=128))
        nc.sync.dma_start(out=bias_sb[:, 4:8], in_=pbeff[:].rearrange("(t p) -> p t", p=128))
        nc.sync.dma_start(out=bias_sb[:, 8:12], in_=nw[:].rearrange("(t p) -> p t", p=128))
        nc.sync.dma_start(out=bias_sb[:, 12:16], in_=nb[:].rearrange("(t p) -> p t", p=128))
        # Absorb the four bias DMA semaphores onto the DVE clock early.
        for j in range(4):
            bt = small.tile([1, 1], F32, tag="btouch", name=f"btouch{j}")
            nc.vector.tensor_copy(out=bt, in_=bias_sb[0:1, 4 * j:4 * j + 1])

        # bf16 staging copies of the selector matrices (entries exact in bf16).
        selbf = singles.tile([128, CT * GPT], BF16)
        selbbf = singles.tile([GPT, CT * 128], BF16)
        nc.vector.tensor_copy(out=selbf, in_=sel_sb)
        nc.vector.tensor_copy(out=selbbf, in_=selb_sb)

        # ---------------- GroupNorm (independent per channel tile) ----------
        # Group aggregation via hi/lo-split bf16 matmuls (exact selector, f32
        # PSUM accumulation) recovers ~fp32 precision.
        for t in range(CT):
            xt = x_sb[:, t * N:(t + 1) * N]
            st = small.tile([128, 2, 6], F32, tag="bn")
            nc.vector.bn_stats(out=st[:, 0, :], in_=xt[:, 0:512])
            nc.vector.bn_stats(out=st[:, 1, :], in_=xt[:, 512:1024])
            mv = small.tile([128, 2], F32, tag="mv")
            nc.vector.bn_aggr(out=mv, in_=st)
            mv2 = small.tile([128, 2], F32, tag="mv2")  # [mean, mean^2 + var]
            nc.vector.tensor_copy(out=mv2[:, 0:1], in_=mv[:, 0:1])
            nc.vector.tensor_scalar(
                out=mv2[:, 1:2], in0=mv[:, 0:1], scalar1=mv[:, 0:1],
                scalar2=mv[:, 1:2], op0=OP.mult, op1=OP.add,
            )
            mv2hi = small.tile([128, 2], BF16, tag="mv2hi")
            nc.vector.tensor_copy(out=mv2hi, in_=mv2)
            mv2lo = small.tile([128, 2], BF16, tag="mv2lo")
            nc.vector.tensor_tensor(out=mv2lo, in0=mv2, in1=mv2hi, op=OP.subtract)
            gps = gnps.tile([GPT, 2], F32, tag="gn", name=f"gps{t}")
            nc.tensor.matmul(
                gps, lhsT=selbf[:, t * GPT:(t + 1) * GPT], rhs=mv2hi,
                start=True, stop=False,
            )
            nc.tensor.matmul(
                gps, lhsT=selbf[:, t * GPT:(t + 1) * GPT], rhs=mv2lo,
                start=False, stop=True,
            )
            m2g = small.tile([GPT, 1], F32, tag="m2g")
            nc.vector.tensor_scalar(
                out=m2g, in0=gps[:, 0:1], scalar1=gps[:, 0:1], scalar2=None,
                op0=OP.mult,
            )
            vvar = small.tile([GPT, 1], F32, tag="vvar")
            nc.vector.tensor_tensor(out=vvar, in0=gps[:, 1:2], in1=m2g, op=OP.subtract)
            sq = small.tile([GPT, 1], F32, tag="sq")
            nc.scalar.activation(
                out=sq, in_=vvar, func=AF.Sqrt, bias=eps_sb[0:GPT], scale=1.0
            )
            gst = small.tile([GPT, 2], F32, tag="gst")  # [M, rstd]
            nc.vector.tensor_copy(out=gst[:, 0:1], in_=gps[:, 0:1])
            nc.vector.reciprocal(out=gst[:, 1:2], in_=sq)
            gsthi = small.tile([GPT, 2], BF16, tag="gsthi")
            nc.vector.tensor_copy(out=gsthi, in_=gst)
            gstlo = small.tile([GPT, 2], BF16, tag="gstlo")
            nc.vector.tensor_tensor(out=gstlo, in0=gst, in1=gsthi, op=OP.subtract)
            gbc = gnps.tile([128, 2], F32, tag="gn", name=f"gbc{t}")
            nc.tensor.matmul(
                gbc, lhsT=selbbf[0:GPT, t * 128:(t + 1) * 128], rhs=gsthi,
                start=True, stop=False,
            )
            nc.tensor.matmul(
                gbc, lhsT=selbbf[0:GPT, t * 128:(t + 1) * 128], rhs=gstlo,
                start=False, stop=True,
            )
            at = ab_sb[:, t:t + 1]
            b2t = ab_sb[:, CT + t:CT + t + 1]
            nc.vector.tensor_scalar(
                out=at, in0=bias_sb[:, 8 + t:9 + t], scalar1=gbc[:, 1:2],
                scalar2=None, op0=OP.mult,
            )
            mtmp = small.tile([128, 1], F32, tag="mtmp")
            nc.vector.tensor_scalar(
                out=mtmp, in0=at, scalar1=gbc[:, 0:1], scalar2=None, op0=OP.mult
            )
            nc.vector.tensor_tensor(
                out=b2t, in0=bias_sb[:, 12 + t:13 + t], in1=mtmp, op=OP.subtract
            )
            nc.vector.tensor_scalar(
                out=y_sb[:, t * N:(t + 1) * N], in0=x_sb[:, t * N:(t + 1) * N],
                scalar1=at, scalar2=b2t, op0=OP.mult, op1=OP.add,
            )

        gn_ctx.close()
        ps = ctx.enter_context(tc.tile_pool(name="ps", bufs=2, space="PSUM"))
        psav = ctx.enter_context(tc.tile_pool(name="psav", bufs=2, space="PSUM"))

        # ---------------- QKV ----------------
        # PE touchers: absorb the weight-DMA semaphores before the matmuls.
        for kt in range(CT):
            nc.tensor.ldweights(weights=wqkv_sb[0:1, kt * W3:kt * W3 + 1])

        def emit_qkv_mt(mt, pool, tag):
            # q/k in [row, n] layout: row-tiles 0..3 -> q, 4..7 -> k
            pp = pool.tile([128, N], F32, tag=tag, name=f"pp{mt}")
            for nh in range(2):
                for kt in range(CT):
                    nc.tensor.matmul(
                        pp[:, nh * 512:(nh + 1) * 512],
                        lhsT=wqkv_sb[:, kt * W3 + mt * 128:kt * W3 + (mt + 1) * 128],
                        rhs=y_sb[:, kt * N + nh * 512:kt * N + (nh + 1) * 512],
                        start=(kt == 0), stop=(kt == CT - 1),
                    )
            if mt < 4:
                nc.vector.tensor_scalar(
                    out=q_sb[:, mt * N:(mt + 1) * N], in0=pp,
                    scalar1=bias_sb[:, mt:mt + 1], scalar2=None, op0=OP.add,
                )
            else:
                km = mt - 4
                nc.vector.tensor_copy(out=k_sb[:, km * N:(km + 1) * N], in_=pp)

        def emit_vt(nt):
            # v directly transposed: [n, vrow], with a ones column per head
            vp = psav.tile([128, 512], F32, tag="av", name=f"vp{nt}")
            for kt in range(CT):
                nc.tensor.matmul(
                    vp,
                    lhsT=y_sb[:, kt * N + nt * 128:kt * N + nt * 128 + 128],
                    rhs=wqkv_sb[:, kt * W3 + 2 * C:kt * W3 + 3 * C],
                    start=(kt == 0), stop=(kt == CT - 1),
                )
            dst = vplus[:, nt * HEADS * VW:(nt + 1) * HEADS * VW]
            dst = dst.rearrange("p (h e) -> p h e", e=VW)[:, :, 0:D]
            nc.vector.tensor_copy(out=dst, in_=vp.rearrange("p (h e) -> p h e", e=D))

        # Only the tiles pair 0 needs up front; the rest interleave with
        # pair 0's attention chunks (they use the idle psav slots).
        emit_qkv_mt(0, ps, "ps")
        emit_qkv_mt(4, ps, "ps")
        deferred = [
            lambda mt=mt: emit_qkv_mt(mt, psav, "av") for mt in (1, 5, 2, 6, 3, 7)
        ] + [lambda nt=nt: emit_vt(nt) for nt in range(NT)]

        # ---------------- Attention (software-pipelined over head pairs) ----
        # Iteration pr emits pair pr's qk+exp chunks interleaved per m-tile
        # with pair pr-1's attention*V matmuls, keeping PE and ACT both busy.
        prev = None  # (pr, heads, etiles, apns)
        for pr in range(5):
            if pr < 4:
                heads = ((2 * pr, 0), (2 * pr + 1, 64))
                etiles = {}
                for h, base in heads:
                    etiles[h] = expp.tile(
                        [128, NT * N], BF16, tag="exp", name=f"exp{h}"
                    )
                apns = {}
                for h, base in heads:
                    apns[h] = psav.tile([D + 1, N], F32, tag="av", name=f"apn{h}")
            for mt in range(NT):
                if pr < 4:
                    # Two heads' qk interleaved: disjoint PE row groups
                    # (0-63 / 64-127) -> adjacent MMs run concurrently.
                    sps = {}
                    for h, base in heads:
                        sps[h] = ps.tile([128, N], F32, tag="ps", name=f"sp{h}_{mt}")
                    for nh in range(2):
                        for h, base in heads:
                            nc.tensor.matmul(
                                sps[h][:, nh * 512:(nh + 1) * 512],
                                lhsT=k_sb[base:base + 64, pr * N + mt * 128:pr * N + mt * 128 + 128],
                                rhs=q_sb[base:base + 64, pr * N + nh * 512:pr * N + nh * 512 + 512],
                                start=True, stop=True,
                                tile_position=(base, 0),
                            )
                    for h, base in heads:
                        nc.scalar.activation(
                            out=etiles[h][:, mt * N:(mt + 1) * N], in_=sps[h],
                            func=AF.Exp, bias=zero_sb, scale=1.0,
                        )
                if pr == 0:
                    for _ in range(2):
                        if deferred:
                            deferred.pop(0)()
                if prev is not None:
                    p_pr, p_heads, p_etiles, p_apns = prev
                    for h, base in p_heads:
                        for nh in range(2):
                            nc.tensor.matmul(
                                p_apns[h][:, nh * 512:(nh + 1) * 512],
                                lhsT=vplus[:, mt * HEADS * VW + h * VW:
                                           mt * HEADS * VW + (h + 1) * VW],
                                rhs=p_etiles[h][:, mt * N + nh * 512:mt * N + nh * 512 + 512],
                                start=(mt == 0), stop=(mt == NT - 1),
                            )
            if prev is not None:
                # Softmax denominators for the drained pair: both Z rows into
                # a [128, 16] layout -> one wide reciprocal (~16 elems/lane),
                # DRAM-bounce broadcast, one DVE multiply per head.
                p_pr, p_heads, p_etiles, p_apns = prev
                zp = small.tile([128, 16], F32, tag="zp", name=f"zp{p_pr}")
                for h, base in p_heads:
                    zrow = small.tile([1, N], F32, tag="zrow", name=f"zrow{h}")
                    nc.vector.tensor_copy(out=zrow, in_=p_apns[h][D:D + 1, :])
                    nc.sync.dma_start(
                        out=zp[:, (h % 2) * 8:(h % 2) * 8 + 8],
                        in_=zrow.rearrange("o (p j) -> o p j", j=8),
                    )
                rzp = small.tile([128, 16], F32, tag="rzp", name=f"rzp{p_pr}")
                nc.vector.reciprocal(out=rzp, in_=zp)
                for h, base in p_heads:
                    zd = drp.tile([N], F32, tag="zd", name=f"zd{h}")
                    nc.sync.dma_start(
                        out=zd, in_=rzp[:, (h % 2) * 8:(h % 2) * 8 + 8]
                    )
                    rzb = work.tile([D, N], F32, tag="rzb")
                    nc.sync.dma_start(
                        out=rzb,
                        in_=bass.AP(tensor=zd.tensor, offset=zd.offset,
                                    ap=[[0, D], [1, N]]),
                    )
                    nc.vector.tensor_tensor(
                        out=av_sb[base:base + 64, p_pr * N:(p_pr + 1) * N],
                        in0=p_apns[h][0:D, :], in1=rzb, op=OP.mult,
                    )
            prev = (pr, heads, etiles, apns) if pr < 4 else None

        # ---------------- Proj + residual ----------------
        for kt in range(CT):
            nc.tensor.ldweights(weights=wproj_sb[0:1, kt * C:kt * C + 1])
        for ct in range(CT):
            pp = ps.tile([128, N], F32, tag="ps")
            for nh in range(2):
                for kt in range(CT):
                    nc.tensor.matmul(
                        pp[:, nh * 512:(nh + 1) * 512],
                        lhsT=wproj_sb[:, kt * C + ct * 128:kt * C + (ct + 1) * 128],
                        rhs=av_sb[:, kt * N + nh * 512:kt * N + nh * 512 + 512],
                        start=(kt == 0), stop=(kt == CT - 1),
                    )
            ob = work.tile([128, N], F32, tag="osb")
            nc.vector.scalar_tensor_tensor(
                out=ob, in0=pp, scalar=bias_sb[:, 4 + ct:5 + ct],
                in1=x_sb[:, ct * N:(ct + 1) * N], op0=OP.add, op1=OP.add,
            )
            nc.sync.dma_start(out=out[ct * 128:(ct + 1) * 128, :], in_=ob)

    return nc


_CACHE = {}


def _get_nc():
    if "nc" not in _CACHE:
        nc = bacc.Bacc()
        _build(nc)
        nc.finalize()
        _CACHE["nc"] = nc
    return _CACHE["nc"]


def prepare_in_maps(x, norm_w, norm_b, qkv_w, qkv_b, proj_w, proj_b):
    x = np.asarray(x, np.float32)
    norm_w = np.asarray(norm_w, np.float32)
    norm_b = np.asarray(norm_b, np.float32)
    qkv_w = np.asarray(qkv_w, np.float32).copy()
    qkv_b = np.asarray(qkv_b, np.float32).copy()
    proj_w = np.asarray(proj_w, np.float32)
    proj_b = np.asarray(proj_b, np.float32)

    scale = D ** -0.5
    qkv_w[:C] *= scale
    qbias = (qkv_b[:C] * scale).astype(np.float32)
    vbias = qkv_b[2 * C:3 * C]
    qkvwT = np.ascontiguousarray(qkv_w.T).astype(ml_dtypes.bfloat16)
    projwT = np.ascontiguousarray(proj_w.T).astype(ml_dtypes.bfloat16)
    pb_eff = (proj_b + proj_w @ vbias).astype(np.float32)

    sel = np.zeros([CT, 128, GPT], np.float32)
    selb = np.zeros([CT, GPT, 128], np.float32)
    for t in range(CT):
        for p in range(128):
            g = p // GSIZE  # group index within this tile
            sel[t, p, g] = 1.0 / GSIZE
            selb[t, g, p] = 1.0
    shared = dict(
        qkvwT=qkvwT, projwT=projwT, qb=qbias, pbeff=pb_eff,
        nw=norm_w, nb=norm_b, sel=sel, selb=selb,
    )
    return [
        dict(x=np.ascontiguousarray(x[i].reshape(C, N)), **shared)
        for i in range(x.shape[0])
    ]


def run(in_maps, trace=False, **kwargs):
    return run_bass_kernel_spmd(
        _get_nc(), in_maps, core_ids=list(range(NCORES)), trace=trace, **kwargs
    )


def kernel(x, norm_w, norm_b, qkv_w, qkv_b, proj_w, proj_b):
    in_maps = prepare_in_maps(x, norm_w, norm_b, qkv_w, qkv_b, proj_w, proj_b)
    res = run(in_maps)
    b, c, h, w = np.asarray(x).shape
    return np.stack(
        [res.results[i]["out"].reshape(c, h, w) for i in range(b)]
    ).astype(np.float32)
